# revision 14
# baseline (speedup 1.0000x reference)
"""KNN-graph (K=2) adjacency kernel for Trainium2, 8 NeuronCores SPMD.

Strategy: shard query rows across 8 cores (2048 rows each). Each core gets a
cyclically-permuted X^T (its own 2048-column band first) so the self-distance
diagonal sits at a core-independent position -> one shared SPMD program.

Per core:
  value[i, j'] = 2*<x_i, x_j> - sq_i - sq_j   (= -dist2, computed by one
  matmul with contraction 66 = 64 features + two augmented rows).
  Self column masked to -1e30; argmax over j' found via a fold-by-halves
  TensorTensorReduce + MaxIndex on the folded array + a half-max test to
  disambiguate which half the argmax came from. Adjacency = zeros (DMA
  zero-fill) + indirect-DMA scatter of 1.0 at (row, self) and (row, argmax).
"""

import os
import sys
import functools

import numpy as np

for _p in ("/opt/trn_rl_repo",):
    if _p not in sys.path and os.path.isdir(_p):
        sys.path.insert(0, _p)

N = 16384
D = 64
NCORES = 8
RPC = N // NCORES          # rows per core = 2048
P = 128                    # partitions / rows per block
NBLK = RPC // P            # 16 blocks per core
NCHUNK = N // 512          # 32 matmul chunks per block
HALF = N // 2              # 8192
WIN = 2048                 # fold window (4 chunks)
NWIN = HALF // WIN         # 4 windows per half
NEG = -3.0e38
DIAG_NEG = -1.0e30


def _body(nc, tc, tile, bass, mybir, adj, xtp, aux, selfoff, hib):
    from contextlib import ExitStack

    f32 = mybir.dt.float32
    u32 = mybir.dt.uint32
    i32 = mybir.dt.int32
    AL = mybir.AluOpType
    AF = mybir.ActivationFunctionType
    X_AX = mybir.AxisListType.X

    ctx = ExitStack()
    with ctx:
        const = ctx.enter_context(tc.tile_pool(name="const", bufs=1))
        aug = ctx.enter_context(tc.tile_pool(name="aug", bufs=1))
        sqp = ctx.enter_context(tc.tile_pool(name="sqp", bufs=3))
        tmps = ctx.enter_context(tc.tile_pool(name="tmps", bufs=4))
        h1p = ctx.enter_context(tc.tile_pool(name="h1p", bufs=10))
        smalls = ctx.enter_context(tc.tile_pool(name="smalls", bufs=2))
        psum = ctx.enter_context(tc.tile_pool(name="psum", bufs=6, space="PSUM"))
        psq = ctx.enter_context(tc.tile_pool(name="psq", bufs=2, space="PSUM"))

        # ---------------- constants ----------------
        zerot = const.tile([P, 2048], f32)
        nc.vector.memset(zerot[:, :], 0.0)
        onesv = const.tile([P, 1], f32)
        nc.vector.memset(onesv[:, :], 1.0)
        ones64 = const.tile([64, 1], f32)
        nc.vector.memset(ones64[:, :], 1.0)

        # diag mask [128,128]: -1e30 on the diagonal, 0 elsewhere
        iod = const.tile([P, P], i32)
        nc.gpsimd.iota(iod[:, :], pattern=[[1, P]], base=0, channel_multiplier=-1)
        eqd = const.tile([P, P], f32)
        nc.vector.tensor_scalar(eqd[:, :], iod[:, :], 0, None, op0=AL.is_equal)
        diagmask = const.tile([P, P], f32)
        nc.vector.tensor_scalar_mul(diagmask[:, :], eqd[:, :], DIAG_NEG)

        # p*16384 as f32 (exact: < 2^21)
        iop = const.tile([P, 1], u32)
        nc.gpsimd.iota(iop[:, :], pattern=[[0, 1]], base=0, channel_multiplier=N)
        iopf = const.tile([P, 1], f32)
        nc.vector.tensor_copy(iopf[:, :], iop[:, :])

        aux_sb = const.tile([P, 3], f32)
        nc.sync.dma_start(aux_sb[:, :], aux[:, :])
        selfoff_sb = const.tile([P, NBLK], u32)
        nc.sync.dma_start(selfoff_sb[:, :], selfoff[:, :])
        hib_sb = const.tile([P, NBLK], mybir.dt.uint16)
        nc.sync.dma_start(hib_sb[:, :], hib[:, :])

        # ---------------- zero-fill the output (4 col-quarters x 16 blocks) --
        adjv = adj.rearrange("(b p) n -> b p n", p=P)
        zf_insts = [[] for _ in range(NBLK)]
        for b in range(NBLK):
            for q in range(8):
                ins = nc.scalar.dma_start(
                    adjv[b, :, q * 2048:(q + 1) * 2048], zerot[:, :]
                )
                zf_insts[b].append(ins)

        # ---------------- augmented operands ----------------
        rhs = aug.tile([66, N], f32)
        nc.sync.dma_start(rhs[0:64, :], xtp[:, :])
        nc.vector.memset(rhs[64:65, :], -1.0)

        lhsT = aug.tile([66, RPC], f32)
        # rows 0-63 = 2 * X^T[:, :2048]
        for k in range(4):
            sl = slice(k * 512, (k + 1) * 512)
            nc.scalar.activation(lhsT[0:64, sl], rhs[0:64, sl], AF.Copy, scale=2.0)
        # rows 64+65 both to -1.0 (base-partition must be 0/32/64/96); the sq
        # DMAs below then overwrite row 64 with +sq.
        nc.vector.memset(lhsT[64:66, :], -1.0)

        # sq_j = sum_d x_jd^2, computed chunkwise; +sq into rhs row 65 and
        # (first 4 chunks) into lhsT row 64.
        for t in range(NCHUNK):
            sl = slice(t * 512, (t + 1) * 512)
            xsq = sqp.tile([64, 512], f32)
            nc.scalar.activation(xsq[:, :], rhs[0:64, sl], AF.Square)
            pq = psq.tile([1, 512], f32)
            nc.tensor.matmul(pq[:, :], lhsT=ones64[:, :], rhs=xsq[:, :],
                             start=True, stop=True)
            tq = tmps.tile([1, 512], f32)
            nc.vector.tensor_copy(tq[:, :], pq[:, :])
            nc.sync.dma_start(rhs[65:66, sl], tq[:, :])
            if t < 4:
                nc.sync.dma_start(lhsT[64:65, sl], tq[:, :])

        # w*2048 per candidate slot, replicated down partitions (f32 exact)
        woffu = const.tile([P, 8], u32)
        nc.gpsimd.iota(woffu[:, :], pattern=[[WIN, 8]], base=0,
                       channel_multiplier=0)
        woff = const.tile([P, 8], f32)
        nc.vector.tensor_copy(woff[:, :], woffu[:, :])

        # ---------------- main loop ----------------
        NWINF = N // WIN  # 8 full-row windows
        for b in range(NBLK):
            lw = lhsT[:, b * P:(b + 1) * P]
            win = [h1p.tile([P, WIN], f32, tag="win", name=f"win_{b}_{w}")
                   for w in range(NWINF)]
            for t in range(NCHUNK):
                ps = psum.tile([P, 512], f32)
                nc.tensor.matmul(ps[:, :], lhsT=lw,
                                 rhs=rhs[:, t * 512:(t + 1) * 512],
                                 start=True, stop=True)
                dst = win[t // 4][:, (t % 4) * 512:(t % 4 + 1) * 512]
                nc.scalar.copy(dst, ps[:, :])

            # mask self-distance (cols [b*128, b*128+128) always in window 0)
            msl = slice(b * P, b * P + P)
            nc.vector.tensor_tensor(win[0][:, msl], win[0][:, msl],
                                    diagmask[:, :], op=AL.add)

            m8 = smalls.tile([P, 8], f32, tag="m8")
            for w in range(NWINF):
                nc.vector.reduce_max(m8[:, w:w + 1], win[w][:, :], axis=X_AX)
            vals8 = smalls.tile([P, 8], f32, tag="vals8")
            nc.vector.max(out=vals8[:, :], in_=m8[:, :])

            candf = smalls.tile([P, 8], f32, tag="candf")
            for w in range(NWINF):
                i8 = smalls.tile([P, 8], u32, tag=f"i8_{w % 2}",
                                 name=f"i8_{b}_{w}")
                nc.vector.max_index(i8[:, :], vals8[:, :], win[w][:, :])
                nc.vector.tensor_copy(candf[:, w:w + 1], i8[:, 0:1])
            # global permuted argmax; not-found windows become ~4.29e9
            nc.vector.tensor_tensor(candf[:, :], candf[:, :], woff[:, :],
                                    op=AL.add)
            jperm = smalls.tile([P, 1], f32, tag="jperm")
            nc.vector.tensor_reduce(jperm[:, :], candf[:, :], axis=X_AX,
                                    op=AL.min)

            # un-rotate: j = j' + rot - 16384*(j' >= thr)
            ge2 = smalls.tile([P, 1], f32, tag="ge2")
            nc.vector.tensor_tensor(ge2[:, :], jperm[:, :], aux_sb[:, 1:2],
                                    op=AL.is_ge)
            nc.vector.tensor_scalar_mul(ge2[:, :], ge2[:, :], -float(N))
            jj = smalls.tile([P, 1], f32, tag="jj")
            nc.vector.tensor_tensor(jj[:, :], jperm[:, :], aux_sb[:, 0:1],
                                    op=AL.add)
            nc.vector.tensor_tensor(jj[:, :], jj[:, :], ge2[:, :], op=AL.add)
            # exact u32 offset row*N + j via u16 halves: the full sum exceeds
            # fp32-exact range, but low16 = 16384*(row%4) + j < 65536 is exact
            # and never carries; high16 = row>>2 comes from a host table.
            lowf = smalls.tile([P, 1], f32, tag="lowf")
            nc.vector.tensor_tensor(lowf[:, :], jj[:, :], aux_sb[:, 2:3],
                                    op=AL.add)
            offu = smalls.tile([P, 1], u32, tag="offu")
            off16 = offu[:, :].bitcast(mybir.dt.uint16)
            nc.vector.tensor_copy(off16[:, 0:1], lowf[:, :])
            nc.vector.tensor_copy(off16[:, 1:2], hib_sb[:, b:b + 1])

            s1 = nc.gpsimd.indirect_dma_start(
                out=adj[:, :],
                out_offset=bass.IndirectOffsetOnAxis(ap=offu[:, 0:1], axis=1),
                in_=onesv[:, 0:1], in_offset=None)
            s2 = nc.gpsimd.indirect_dma_start(
                out=adj[:, :],
                out_offset=bass.IndirectOffsetOnAxis(ap=selfoff_sb[:, b:b + 1],
                                                     axis=1),
                in_=onesv[:, 0:1], in_offset=None)
            try:
                from concourse.tile_rust import add_dep_helper
                for zi in zf_insts[b]:
                    for si in (s1, s2):
                        add_dep_helper(getattr(si, "ins", si),
                                       getattr(zi, "ins", zi),
                                       sync=True,
                                       reason="scatter after zero-fill")
            except Exception:
                pass


@functools.cache
def _build():
    import concourse.bass as bass
    import concourse.tile as tile
    from concourse import bacc, mybir

    nc = bacc.Bacc("TRN2", target_bir_lowering=False, debug=False,
                   num_devices=NCORES)
    xtp = nc.dram_tensor("xtp", [D, N], mybir.dt.float32,
                         kind="ExternalInput").ap()
    aux = nc.dram_tensor("aux", [P, 3], mybir.dt.float32,
                         kind="ExternalInput").ap()
    selfoff = nc.dram_tensor("selfoff", [P, NBLK], mybir.dt.uint32,
                             kind="ExternalInput").ap()
    hib = nc.dram_tensor("hib", [P, NBLK], mybir.dt.uint16,
                         kind="ExternalInput").ap()
    adj = nc.dram_tensor("adj", [RPC, N], mybir.dt.float32,
                         kind="ExternalOutput").ap()
    with tile.TileContext(nc) as tc:
        _body(nc, tc, tile, bass, mybir, adj, xtp, aux, selfoff, hib)
    nc.compile()
    return nc


def _in_maps(X):
    XT = np.ascontiguousarray(X.T.astype(np.float32, copy=False))  # [64, N]
    maps = []
    pcol = np.arange(P, dtype=np.int64)
    bcol = np.arange(NBLK, dtype=np.int64)
    for c in range(NCORES):
        rot = c * RPC
        xtp = np.ascontiguousarray(np.roll(XT, -rot, axis=1))
        aux = np.zeros((P, 3), np.float32)
        aux[:, 0] = rot
        aux[:, 1] = N - rot
        aux[:, 2] = (pcol % 4) * N
        # full flat element offset of the self cell: row-local*(N) + self col
        rows = bcol[None, :] * P + pcol[:, None]
        so = (rows * N + (rot + rows)).astype(np.uint32)
        hib = (rows >> 2).astype(np.uint16)
        maps.append({"xtp": xtp, "aux": aux, "selfoff": so, "hib": hib})
    return maps


def run(X, **kwargs):
    """Build+run; returns (adjacency [N,N] f32, BassKernelResults)."""
    from concourse import bass_utils
    nc = _build()
    res = bass_utils.run_bass_kernel_spmd(nc, _in_maps(np.asarray(X)),
                                          core_ids=list(range(NCORES)),
                                          **kwargs)
    out = np.concatenate([r["adj"] for r in res.results], axis=0)
    return out, res


def kernel(X):
    out, _ = run(X)
    return out.astype(np.float32, copy=False)


if __name__ == "__main__":
    rng = np.random.default_rng(0)
    X = rng.standard_normal((N, D)).astype(np.float32)
    out = kernel(X)
    print("out", out.shape, out.dtype, "row sums", out.sum(1)[:8])
